# revision 1
# baseline (speedup 1.0000x reference)
"""Causal multi-head attention on 8 Trainium2 NeuronCores.

Full module: x:(2,2048,1024) f32, 16 heads, head_dim 64, causal softmax
(scaled by 1/sqrt(1024)), out = attn(x) @ Wo + bo.

Sharding: core c handles batch b = c // 4 and head group g = c % 4
(4 heads, i.e. 256 columns of Wq/Wk/Wv and 256 rows of Wo). Every core
runs the same program (SPMD); the host sums the 4 per-group partial
output projections per batch and adds the bias.

Per-core kernel layout strategy (all matmuls contract over the SBUF
partition dim; "T" tensors are stored feature-major so no transposes are
needed mid-attention):
  xT   [f=1024, t=2048]  bf16, built via PE transposes + cast on evict
  QT/KT[d=256,  t=2048]  = Wq/Wk as lhsT, xT as rhs  (2 tiles, head pairs)
  V    [t=2048, d=256]   = xT as lhsT, Wv as rhs; stored [128,16,4,65]
                          with a ones column per head (fused softmax sums)
  S^T  [k=128, q=512]    = KT-slice as lhsT, QT-slice as rhs, two heads
                          run on disjoint partition halves (row-tiled)
  P^T  = exp(S^T/32) via ScalarE, causal mask applied on diagonal blocks
  ctxT [d=64|sums, q]    = [V|1] as lhsT, P^T as rhs, accumulated in PSUM
  norm: recip(sums row) -> PE outer-product broadcast -> DVE multiply
  out  [t, 1024]         = ctxT as lhsT, Wo as rhs  (partial; host-summed)
"""

import os

import numpy as np

N = 2048        # tokens per batch
D = 1024        # model dim
HG = 4          # heads per core
HD = 64         # head dim
DG = HG * HD    # 256, feature columns per core
SCALE = 1.0 / 32.0  # 1/sqrt(D); note module scales by sqrt(d_out), not head_dim
NCORES = 8

# tuning knobs (env-overridable for experiments)
OUTER_F32R = os.environ.get("OUTER_F32R", "1") != "0"  # f32r outer products
PT_BUFS = int(os.environ.get("PT_BUFS", "8"))
HOST_XT = os.environ.get("HOST_XT", "0") != "0"      # host passes x already transposed

_CACHE = {}


def _build_nc(repeat=1):
    from contextlib import ExitStack

    import concourse.mybir as mybir
    import concourse.tile as tile
    from concourse import bacc
    from concourse.masks import make_identity

    FP32 = mybir.dt.float32
    F32R = mybir.dt.float32r
    BF16 = mybir.dt.bfloat16
    EXP = mybir.ActivationFunctionType.Exp
    COPY = mybir.ActivationFunctionType.Copy

    NT = N // 128   # 16 token chunks
    NF = D // 128   # 8 feature chunks
    NQ = N // 512   # 4 query blocks

    nc = bacc.Bacc("TRN2", target_bir_lowering=False, debug=False)

    if HOST_XT:
        x_d = nc.dram_tensor("x", [D, N], BF16, kind="ExternalInput").ap()
    else:
        x_d = nc.dram_tensor("x", [N, D], FP32, kind="ExternalInput").ap()
    wq_d = nc.dram_tensor("wq", [D, DG], BF16, kind="ExternalInput").ap()
    wk_d = nc.dram_tensor("wk", [D, DG], BF16, kind="ExternalInput").ap()
    wv_d = nc.dram_tensor("wv", [D, DG], BF16, kind="ExternalInput").ap()
    wo_d = nc.dram_tensor("wo", [DG, D], BF16, kind="ExternalInput").ap()
    out_d = nc.dram_tensor("out", [N, D], FP32, kind="ExternalOutput").ap()

    with tile.TileContext(nc) as tc, ExitStack() as ctx:
        persist = ctx.enter_context(tc.tile_pool(name="persist", bufs=1))
        xpool = ctx.enter_context(tc.tile_pool(name="xpool", bufs=10))
        ptpool = ctx.enter_context(tc.tile_pool(name="ptpool", bufs=PT_BUFS))
        stpool = ctx.enter_context(tc.tile_pool(name="stpool", bufs=4))
        smpool = ctx.enter_context(tc.tile_pool(name="smpool", bufs=4))
        opool = ctx.enter_context(tc.tile_pool(name="opool", bufs=3))
        # PSUM budget (8 banks): proj/transpose/outproj tag "ps"
        # [128,512]x2 = 2 banks; attention S tag "ps_s" [128,1024]x2 = 4
        # banks (independent rotation domains so the phases pipeline);
        # pv pool 2 banks, slots reused for the broadcast outer-products.
        mmpsum = ctx.enter_context(tc.tile_pool(name="mmpsum", bufs=2, space="PSUM"))
        spsum = mmpsum
        pvpsum = ctx.enter_context(tc.tile_pool(name="pvpsum", bufs=1, space="PSUM"))

        # ---- persistent tensors ----
        xT = persist.tile([128, NF, N], BF16, name="xT")          # 32 KB/p
        qt = persist.tile([128, 2, N], BF16, name="qt")           # 8 KB/p
        kt = persist.tile([128, 2, N], BF16, name="kt")           # 8 KB/p
        vt = persist.tile([128, NT, HG, HD + 1], BF16, name="vt")  # ~8 KB/p
        ctxT = persist.tile([128, 2, N], BF16, name="ctxT")       # 8 KB/p
        wq_bf = persist.tile([128, NF, DG], BF16, name="wq_bf")   # 4 KB/p
        wk_bf = persist.tile([128, NF, DG], BF16, name="wk_bf")
        wv_bf = persist.tile([128, NF, DG], BF16, name="wv_bf")
        wo_bf = persist.tile([128, 2, D], BF16, name="wo_bf")     # 4 KB/p
        ones128 = persist.tile([128, HD], F32R if OUTER_F32R else FP32,
                               name="ones128")
        if not HOST_XT:
            ident = persist.tile([128, 128], FP32, name="ident")
            make_identity(nc, ident[:, :])

        if OUTER_F32R:
            # walrus requires f32r operands produced by a rounding op
            ones_f32 = persist.tile([128, HD], FP32, name="ones_f32")
            nc.gpsimd.memset(ones_f32[:, :], 1.0)
            nc.vector.tensor_copy(ones128[:, :], ones_f32[:, :])
        else:
            nc.gpsimd.memset(ones128[:, :], 1.0)
        nc.gpsimd.memset(vt[:, :, :, HD], 1.0)  # softmax-sum ones columns

        def emit_weights():
            # ---- weights arrive bf16 from the host: straight DMAs ----
            for w_dram, w_bf in ((wq_d, wq_bf), (wk_d, wk_bf), (wv_d, wv_bf)):
                nc.sync.dma_start(out=w_bf[:, :, :],
                                  in_=w_dram.rearrange("(c p) d -> p c d", p=128))
            nc.sync.dma_start(out=wo_bf[:, :, :],
                              in_=wo_d.rearrange("(c p) d -> p c d", p=128))

        def emit_xt_block(ib):
            """Build the bf16 xT slab for one 512-token range."""
            if HOST_XT:
                # x arrives feature-major in bf16: straight DMA into xT
                nc.sync.dma_start(
                    out=xT[:, :, 512 * ib:512 * (ib + 1)],
                    in_=x_d.rearrange("(c p) t -> p c t", p=128)[
                        :, :, 512 * ib:512 * (ib + 1)],
                )
                return
            for u in range(4):
                ti = 4 * ib + u
                t0 = ti * 128
                xn_t = xpool.tile([128, D], FP32, name="xn")
                nc.sync.dma_start(out=xn_t[:, :], in_=x_d[t0:t0 + 128, :])
                for jh in range(2):        # f chunks [4jh .. 4jh+3]
                    ps_tr = mmpsum.tile([128, 512], FP32, name="ps",
                                        tag="ps")
                    for v in range(4):
                        j = 4 * jh + v
                        nc.tensor.transpose(
                            ps_tr[:, 128 * v:128 * (v + 1)],
                            xn_t[:, 128 * j:128 * (j + 1)],
                            ident[:, :],
                        )
                    nc.vector.tensor_copy(
                        xT[:, 4 * jh:4 * jh + 4, 128 * ti:128 * (ti + 1)],
                        ps_tr[:, :].rearrange("p (j t) -> p j t", j=4))

        def emit_proj_mms(ib):
            """Project one 512-token range of xT into QT/KT/V."""
            tb = ib
            for w_bf, dst in ((wq_bf, qt), (wk_bf, kt)):
                for dh in range(2):
                    ps = mmpsum.tile([128, 512], FP32, name="ps", tag="ps")
                    for fc in range(NF):
                        nc.tensor.matmul(
                            ps[:, :],
                            lhsT=w_bf[:, fc, 128 * dh:128 * (dh + 1)],
                            rhs=xT[:, fc, 512 * tb:512 * (tb + 1)],
                            start=(fc == 0), stop=(fc == NF - 1),
                        )
                    nc.vector.tensor_copy(
                        dst[:, dh, 512 * tb:512 * (tb + 1)], ps[:, :])
            for tcc in range(4 * ib, 4 * ib + 4):
                ps = mmpsum.tile([128, 512], FP32, name="ps", tag="ps")
                for fc in range(NF):
                    nc.tensor.matmul(
                        ps[:, 0:DG],
                        lhsT=xT[:, fc, 128 * tcc:128 * (tcc + 1)],
                        rhs=wv_bf[:, fc, :],
                        start=(fc == 0), stop=(fc == NF - 1),
                    )
                nc.vector.tensor_copy(
                    vt[:, tcc, :, 0:HD],
                    ps[:, 0:DG].rearrange("p (h e) -> p h e", h=HG))

        def emit_attention(qb):
            """Attention for one 512-wide query block, both head pairs,
            then the output projection for the same token range."""
            nkc = 4 * (qb + 1)             # causal: k chunks 0..4qb+3
            qsl = slice(512 * qb, 512 * (qb + 1))
            for p in range(2):             # head pair (heads 2p, 2p+1)
                # S + exp + PV stream (PV lags one chunk behind exp)
                pv_a = pvpsum.tile([HD + 1, 512], FP32, name="pv_a",
                                   tag="pv_a")
                pv_b = pvpsum.tile([HD + 1, 512], FP32, name="pv_b",
                                   tag="pv_b")
                for kc in range(nkc):
                    ksl = slice(128 * kc, 128 * (kc + 1))
                    # columns q_local < 128*m are entirely above the
                    # causal diagonal for this k chunk: skip them.
                    m = max(0, kc - 4 * qb)
                    q0 = 128 * m
                    ps_s = spsum.tile([128, 1024], FP32, name="ps_s",
                                      tag="ps_s", bufs=2)
                    # head A on partitions 0-63, head B on 64-127
                    for i in range(2):
                        lo = 64 * i
                        nc.tensor.matmul(
                            ps_s[:, 512 * i:512 * (i + 1)],
                            lhsT=kt[lo:lo + 64, p, ksl],
                            rhs=qt[lo:lo + 64, p, qsl],
                            start=True, stop=True,
                        )
                    pt = ptpool.tile([128, 1024], BF16, name="pt")
                    # full-width exp even on narrowed diagonal chunks: the
                    # skipped columns hold stale PSUM (finite, O(1)) and
                    # are never read downstream
                    nc.scalar.activation(pt[:, :], ps_s[:, :], EXP,
                                         scale=SCALE)
                    if kc >= 4 * qb:       # diagonal: zero q < k in
                        # place on the columns PV will actually read
                        for i in range(2):
                            sl = slice(512 * i + q0, 512 * (i + 1))
                            nc.gpsimd.affine_select(
                                out=pt[:, sl], in_=pt[:, sl],
                                compare_op=mybir.AluOpType.is_ge,
                                fill=0.0,
                                base=0,
                                pattern=[[1, 512 - q0]],
                                channel_multiplier=-1,
                            )
                    st = (kc == 0)
                    sp = (kc == nkc - 1)
                    nc.tensor.matmul(
                        pv_a[:, q0:512], lhsT=vt[:, kc, 2 * p, :],
                        rhs=pt[:, q0:512], start=st, stop=sp,
                    )
                    nc.tensor.matmul(
                        pv_b[:, q0:512], lhsT=vt[:, kc, 2 * p + 1, :],
                        rhs=pt[:, 512 + q0:1024], start=st, stop=sp,
                    )
                # epilogue: stage PSUM out (frees pv slots for the bc
                # outer-products), then normalize by the fused sums row
                st_a = stpool.tile([HD + 1, 512], FP32, name="st_a", tag="st")
                st_b = stpool.tile([HD + 1, 512], FP32, name="st_b", tag="st")
                nc.vector.tensor_copy(st_a[:, :], pv_a[:, :])
                nc.vector.tensor_copy(st_b[:, :], pv_b[:, :])
                rec = smpool.tile([HD + 1, 1024],
                                  F32R if OUTER_F32R else FP32, name="rec")
                with nc.allow_low_precision(reason="f32r softmax recip"):
                    nc.vector.reciprocal(rec[HD:HD + 1, 0:512],
                                         st_a[HD:HD + 1, :])
                    nc.vector.reciprocal(rec[HD:HD + 1, 512:1024],
                                         st_b[HD:HD + 1, :])
                bc_a = pvpsum.tile([HD, 512], FP32, name="bc_a", tag="pv_a")
                bc_b = pvpsum.tile([HD, 512], FP32, name="bc_b", tag="pv_b")
                ones_ap = ones128[HD:HD + 1, :]
                rec_a = rec[HD:HD + 1, 0:512]
                rec_b = rec[HD:HD + 1, 512:1024]
                nc.tensor.matmul(bc_a[:, :], lhsT=ones_ap, rhs=rec_a,
                                 start=True, stop=True)
                nc.tensor.matmul(bc_b[:, :], lhsT=ones_ap, rhs=rec_b,
                                 start=True, stop=True)
                # head A lands on ctxT partitions 0-63 directly
                nc.vector.tensor_mul(ctxT[0:HD, p, qsl], st_a[0:HD, :],
                                     bc_a[:, :])
                # head B: multiply at partitions 0-63, DMA to 64-127
                cb = stpool.tile([HD, 512], BF16, name="cb", tag="cb")
                nc.vector.tensor_mul(cb[:, :], st_b[0:HD, :], bc_b[:, :])
                nc.sync.dma_start(out=ctxT[HD:128, p, qsl], in_=cb[:, :])
            # output projection for this token range (partial over heads)
            for tb in range(4 * qb, 4 * qb + 4):
                tsl = slice(128 * tb, 128 * (tb + 1))
                for nh in range(2):
                    ps_o = mmpsum.tile([128, 512], FP32, name="ps", tag="ps")
                    for hc in range(2):
                        nc.tensor.matmul(
                            ps_o[:, :],
                            lhsT=ctxT[:, hc, tsl],
                            rhs=wo_bf[:, hc, 512 * nh:512 * (nh + 1)],
                            start=(hc == 0), stop=(hc == 1),
                        )
                    o_sb = opool.tile([128, 512], FP32, name="o_sb")
                    # last query block: exps are done by then, ScalarE idle
                    if nh == 0 and qb != NQ - 1:
                        nc.vector.tensor_copy(o_sb[:, :], ps_o[:, :])
                    else:
                        nc.scalar.activation(o_sb[:, :], ps_o[:, :], COPY)
                    nc.sync.dma_start(
                        out=out_d[tsl, 512 * nh:512 * (nh + 1)],
                        in_=o_sb[:, :])

        def emit_body():
            emit_xt_block(0)
            emit_weights()
            emit_proj_mms(0)
            emit_xt_block(1)
            emit_proj_mms(1)
            emit_attention(0)
            emit_xt_block(2)
            emit_proj_mms(2)
            emit_attention(1)
            emit_xt_block(3)
            emit_proj_mms(3)
            emit_attention(2)
            emit_attention(3)

        for _rep in range(repeat):
            emit_body()

    nc.compile()
    return nc


def _get_nc(repeat=1):
    key = ("nc", repeat)
    if key not in _CACHE:
        _CACHE[key] = _build_nc(repeat)
    return _CACHE[key]


def _make_in_maps(x, Wq, Wk, Wv, Wo):
    in_maps = []
    for c in range(NCORES):
        b, g = divmod(c, 4)
        cs = slice(DG * g, DG * (g + 1))
        if HOST_XT:
            import ml_dtypes
            xb = np.ascontiguousarray(x[b].T).astype(ml_dtypes.bfloat16)
        else:
            xb = np.ascontiguousarray(x[b], dtype=np.float32)
        import ml_dtypes
        bf = ml_dtypes.bfloat16
        in_maps.append({
            "x": xb,
            "wq": np.ascontiguousarray(Wq[:, cs]).astype(bf),
            "wk": np.ascontiguousarray(Wk[:, cs]).astype(bf),
            "wv": np.ascontiguousarray(Wv[:, cs]).astype(bf),
            "wo": np.ascontiguousarray(Wo[cs, :]).astype(bf),
        })
    return in_maps


def _gather(results, bo):
    out = np.empty((2, N, D), dtype=np.float32)
    for b in range(2):
        acc = results[4 * b]["out"].astype(np.float32)
        for g in range(1, 4):
            acc = acc + results[4 * b + g]["out"]
        out[b] = acc + bo[None, :].astype(np.float32)
    return out


def run_spmd(x, Wq, Wk, Wv, Wo, bo, **spmd_kwargs):
    """Run the 8-core kernel; returns (full_output, BassKernelResults)."""
    from concourse.bass_utils import run_bass_kernel_spmd

    nc = _get_nc()
    in_maps = _make_in_maps(
        np.asarray(x), np.asarray(Wq), np.asarray(Wk), np.asarray(Wv),
        np.asarray(Wo))
    res = run_bass_kernel_spmd(nc, in_maps, core_ids=list(range(NCORES)),
                               **spmd_kwargs)
    return _gather(res.results, np.asarray(bo)), res


def kernel(x, Wq, Wk, Wv, Wo, bo):
    out, _ = run_spmd(x, Wq, Wk, Wv, Wo, bo)
    return out



# revision 17
# speedup vs baseline: 3.0355x; 3.0355x over previous
"""Causal multi-head attention on 8 Trainium2 NeuronCores.

Full module: x:(2,2048,1024) f32, 16 heads, head_dim 64, causal softmax
(scaled by 1/sqrt(1024)), out = attn(x) @ Wo + bo.

Sharding: core c handles batch b = c // 4 and head group g = c % 4
(4 heads = 2 head pairs, i.e. 256 columns of Wq/Wk/Wv and 256 rows of
Wo). Every core runs the same program (SPMD); the host sums the 4
per-group partial output projections per batch and adds the bias.

Per-core layout (all matmuls contract over the SBUF partition dim):
  xT   [f=1024, t=2048] bf16, transposed on the host, straight DMA in
  QT/KT[d, t] per head pair: partitions = d within pair (head A 0-63,
       head B 64-127), built from Wq/Wk as lhsT against xT
  V    [t, d] token-major [128, 16 chunks, 4 heads, 64+1] with a fused
       ones column per head (softmax sums fall out of the PV matmul)
  S^T  [k=128, q<=512] per chunk = KT-slice lhsT x QT rhs, two heads on
       disjoint 64-partition halves, diagonal chunks narrowed to the
       causal column range
  P^T  = exp(S^T/32) on ScalarE; diagonal 128-block masked by a bf16
       lower-triangle multiply on DVE
  ctx  [q=128, 65] per query sub-chunk = pt-slice lhsT x V rhs,
       accumulated over k chunks in PSUM; col 64 = softmax sums
  norm: reciprocal(sums) -> per-partition tensor_scalar multiply (DVE)
  ctxT via PE transpose of the normalized [q, d] tile
  out  [t, 1024] = ctxT lhsT x Wo rhs (partial over heads; host sums)

Emission is software-pipelined: the 80 S/exp chunk units pace the body
and every other PE-side unit (projections of later token blocks, PV,
transposes, output projection) is interleaved between chunks with a
small lag so no engine queue head-blocks on ScalarE.
"""

import os

import numpy as np

N = 2048        # tokens per batch
D = 1024        # model dim
HG = 4          # heads per core
HD = 64         # head dim
DG = HG * HD    # 256, feature columns per core
SCALE = 1.0 / 32.0  # 1/sqrt(D); module scales by sqrt(d_out), not head_dim
NCORES = 8

PT_BUFS = int(os.environ.get("PT_BUFS", "22"))
# gpsimd cannot read PSUM (BIR verifier) — drains stay on DVE
GPSIMD_DRAIN = os.environ.get("GPSIMD_DRAIN", "0") != "0"
# fp8 (e4m3) Q/K projections via DoubleRow matmuls (2x PE throughput).
# Wq/Wk are pre-scaled by W8SCALE into fp8 range; the exp() scale divides
# the resulting alpha^2 factor back out of the scores.
FP8_QK = os.environ.get("FP8_QK", "0") != "0"
W8SCALE = 64.0

_CACHE = {}


def _build_nc(repeat=1):
    from contextlib import ExitStack

    import concourse.mybir as mybir
    import concourse.tile as tile
    from concourse import bacc

    FP32 = mybir.dt.float32
    BF16 = mybir.dt.bfloat16
    FP8 = mybir.dt.float8e4
    EXP = mybir.ActivationFunctionType.Exp
    DR = mybir.MatmulPerfMode.DoubleRow

    NT = N // 128   # 16 token chunks
    NF = D // 128   # 8 feature chunks
    NQ = N // 512   # 4 query blocks

    nc = bacc.Bacc("TRN2", target_bir_lowering=False, debug=False)

    x_d = nc.dram_tensor("x", [D, N], BF16, kind="ExternalInput").ap()
    if FP8_QK:
        x8_d = nc.dram_tensor("x8", [D, N], FP8, kind="ExternalInput").ap()
        wq_d = nc.dram_tensor("wq", [D, DG], FP8, kind="ExternalInput").ap()
        wk_d = nc.dram_tensor("wk", [D, DG], FP8, kind="ExternalInput").ap()
    else:
        wq_d = nc.dram_tensor("wq", [D, DG], BF16, kind="ExternalInput").ap()
        wk_d = nc.dram_tensor("wk", [D, DG], BF16, kind="ExternalInput").ap()
    wv_d = nc.dram_tensor("wv", [D, DG], BF16, kind="ExternalInput").ap()
    wo_d = nc.dram_tensor("wo", [DG, D], BF16, kind="ExternalInput").ap()
    # partial outputs leave in bf16; the host sums the 4 groups in f32
    out_d = nc.dram_tensor("out", [N, D], BF16, kind="ExternalOutput").ap()

    with tile.TileContext(nc) as tc, ExitStack() as ctx:
        persist = ctx.enter_context(tc.tile_pool(name="persist", bufs=1))
        ptpool = ctx.enter_context(tc.tile_pool(name="ptpool", bufs=PT_BUFS))
        smpool = ctx.enter_context(tc.tile_pool(name="smpool", bufs=4))
        cspool = ctx.enter_context(tc.tile_pool(name="cspool", bufs=4))
        opool = ctx.enter_context(tc.tile_pool(name="opool", bufs=3))
        # PSUM (8 banks): "ps" [128,512] x2 (proj drains, ctx transposes,
        # out-proj) = 2; "ps_s" [128,2,512] x2 (S chunks) = 4; "ctx"
        # [128,2,65] x2 (PV accumulation) = 2.
        mmpsum = ctx.enter_context(tc.tile_pool(name="mmpsum", bufs=2, space="PSUM"))
        spsum = ctx.enter_context(tc.tile_pool(name="spsum", bufs=2, space="PSUM"))
        cpsum = ctx.enter_context(tc.tile_pool(name="cpsum", bufs=2, space="PSUM"))

        # ---- persistent tensors ----
        xT = persist.tile([128, NF, N], BF16, name="xT")          # 32 KB/p
        qt = persist.tile([128, 2, N], BF16, name="qt")           # 8 KB/p
        kt = persist.tile([128, 2, N], BF16, name="kt")           # 8 KB/p
        vt = persist.tile([128, NT, HG, HD + 1], BF16, name="vt")  # ~8 KB/p
        ctxT = persist.tile([128, 2, N], BF16, name="ctxT")       # 8 KB/p
        if FP8_QK:
            xT8 = persist.tile([128, NF, N], FP8, name="xT8")     # 16 KB/p
            # [f-pair, k-tile, head, d]: lhsT slice for one DoubleRow matmul
            wq_bf = persist.tile([128, NF // 2, 2, HG, HD], FP8, name="wq_f8")
            wk_bf = persist.tile([128, NF // 2, 2, HG, HD], FP8, name="wk_f8")
        else:
            wq_bf = persist.tile([128, NF, DG], BF16, name="wq_bf")  # 2 KB/p
            wk_bf = persist.tile([128, NF, DG], BF16, name="wk_bf")
        wv_bf = persist.tile([128, NF, DG], BF16, name="wv_bf")
        wo_bf = persist.tile([128, 2, D], BF16, name="wo_bf")     # 4 KB/p
        ident = persist.tile([128, 128], BF16, name="ident")
        cmask = persist.tile([128, 128], BF16, name="cmask")

        # identity (for ctx transposes) and causal mask, built once
        nc.gpsimd.memset(ident[:, :], 0.0)
        nc.gpsimd.affine_select(
            out=ident[:, :], in_=ident[:, :],
            compare_op=mybir.AluOpType.not_equal, fill=1.0,
            base=0, pattern=[[1, 128]], channel_multiplier=-1,
        )
        # cmask[k, q] = 1 where q >= k (keep at/below diagonal of P^T)
        nc.gpsimd.memset(cmask[:, :], 1.0)
        nc.gpsimd.affine_select(
            out=cmask[:, :], in_=cmask[:, :],
            compare_op=mybir.AluOpType.is_ge, fill=0.0,
            base=0, pattern=[[1, 128]], channel_multiplier=-1,
        )
        nc.gpsimd.memset(vt[:, :, :, HD], 1.0)  # softmax-sum ones columns

        def emit_weights_qkv():
            if FP8_QK:
                for w_dram, w8 in ((wq_d, wq_bf), (wk_d, wk_bf)):
                    nc.sync.dma_start(
                        out=w8[:, :, :, :, :],
                        in_=w_dram.rearrange("(a b p) (h e) -> p a b h e",
                                             a=NF // 2, b=2, p=128, h=HG))
                nc.sync.dma_start(out=wv_bf[:, :, :],
                                  in_=wv_d.rearrange("(c p) d -> p c d", p=128))
                return
            for w_dram, w_bf in ((wq_d, wq_bf), (wk_d, wk_bf), (wv_d, wv_bf)):
                nc.sync.dma_start(out=w_bf[:, :, :],
                                  in_=w_dram.rearrange("(c p) d -> p c d", p=128))

        def emit_weights_o():
            nc.sync.dma_start(out=wo_bf[:, :, :],
                              in_=wo_d.rearrange("(c p) d -> p c d", p=128))

        def emit_xt_dma(ib):
            nc.sync.dma_start(
                out=xT[:, :, 512 * ib:512 * (ib + 1)],
                in_=x_d.rearrange("(c p) t -> p c t", p=128)[
                    :, :, 512 * ib:512 * (ib + 1)],
            )
            if FP8_QK:
                nc.sync.dma_start(
                    out=xT8[:, :, 512 * ib:512 * (ib + 1)],
                    in_=x8_d.rearrange("(c p) t -> p c t", p=128)[
                        :, :, 512 * ib:512 * (ib + 1)],
                )

        def proj_units(ib):
            """Projection of one 512-token block as 8 filler units."""
            tb = ib
            units = []
            for w_bf, dst in ((wq_bf, qt), (wk_bf, kt)):
                for dh in range(2):
                    def u(w_bf=w_bf, dst=dst, dh=dh):
                        ps = mmpsum.tile([128, 512], FP32, name="ps", tag="ps")
                        for fc in range(NF):
                            nc.tensor.matmul(
                                ps[:, :],
                                lhsT=w_bf[:, fc, 128 * dh:128 * (dh + 1)],
                                rhs=xT[:, fc, 512 * tb:512 * (tb + 1)],
                                start=(fc == 0), stop=(fc == NF - 1),
                            )
                        nc.vector.tensor_copy(
                            dst[:, dh, 512 * tb:512 * (tb + 1)], ps[:, :])
                    units.append(u)
            for tcc in range(4 * ib, 4 * ib + 4):
                def u(tcc=tcc):
                    ps = mmpsum.tile([128, 512], FP32, name="ps", tag="ps")
                    for fc in range(NF):
                        nc.tensor.matmul(
                            ps[:, 0:DG],
                            lhsT=xT[:, fc, 128 * tcc:128 * (tcc + 1)],
                            rhs=wv_bf[:, fc, :],
                            start=(fc == 0), stop=(fc == NF - 1),
                        )
                    nc.vector.tensor_copy(
                        vt[:, tcc, :, 0:HD],
                        ps[:, 0:DG].rearrange("p (h e) -> p h e", h=HG))
                units.append(u)
            return units

        def emit_chunk(qb, p, kc, pts):
            """One S+exp(+mask) chunk for query block qb, head pair p."""
            ksl = slice(128 * kc, 128 * (kc + 1))
            m = max(0, kc - 4 * qb)
            q0 = 128 * m
            ps_s = spsum.tile([128, 2, 512], FP32, name="ps_s", tag="ps_s")
            for i in range(2):
                lo = 64 * i
                nc.tensor.matmul(
                    ps_s[:, i, q0:512],
                    lhsT=kt[lo:lo + 64, p, ksl],
                    rhs=qt[lo:lo + 64, p, 512 * qb + q0:512 * (qb + 1)],
                    start=True, stop=True,
                )
            pt = ptpool.tile([128, 2, 512], BF16, name="pt", tag="pt")
            nc.scalar.activation(pt[:, :, q0:512], ps_s[:, :, q0:512], EXP,
                                 scale=SCALE)
            if kc >= 4 * qb:    # diagonal chunk: mask the [q0, q0+128) block
                for i in range(2):
                    nc.vector.tensor_mul(pt[:, i, q0:q0 + 128],
                                         pt[:, i, q0:q0 + 128], cmask[:, :])
            pts.append(pt)

        def pv_unit(qb, p, u, pts, trs):
            """PV + normalize for query sub-chunk u of (qb, p)."""
            nkk = 4 * qb + u + 1
            ctx_ps = cpsum.tile([128, 2, HD + 1], FP32, name="ctx_ps",
                                tag="ctx")
            for i in range(2):
                for kc in range(nkk):
                    nc.tensor.matmul(
                        ctx_ps[:, i, :],
                        lhsT=pts[kc][:, i, 128 * u:128 * (u + 1)],
                        rhs=vt[:, kc, 2 * p + i, :],
                        start=(kc == 0), stop=(kc == nkk - 1),
                    )
            rec = smpool.tile([128, 2], FP32, name="rec")
            with nc.allow_low_precision(reason="softmax reciprocal"):
                nc.vector.reciprocal(rec[:, :], ctx_ps[:, :, HD])
            ctx_sb = cspool.tile([128, 2, HD], BF16, name="ctx_sb")
            for i in range(2):
                nc.vector.tensor_scalar_mul(
                    ctx_sb[:, i, :], ctx_ps[:, i, 0:HD], rec[:, i:i + 1])
            trs[u] = ctx_sb

        def t_unit(qb, p, trs):
            """Transpose the 4 normalized ctx tiles into ctxT."""
            qsl = slice(512 * qb, 512 * (qb + 1))
            ps_tr = mmpsum.tile([128, 512], BF16, name="ps_tr", tag="ps")
            for u in range(4):
                nc.tensor.transpose(
                    ps_tr[:, 128 * u:128 * (u + 1)],
                    trs[u][:, :, :].rearrange("p i e -> p (i e)"),
                    ident[:, :],
                )
            nc.vector.tensor_copy(ctxT[:, p, qsl], ps_tr[:, :])

        def outproj_units(qb):
            units = []
            for tb in range(4 * qb, 4 * qb + 4):
                tsl = slice(128 * tb, 128 * (tb + 1))
                for nh in range(2):
                    # alternate drains between Pool and DVE so neither
                    # paces the 2-slot psum rotation; the trailing qb=3
                    # block goes all-DVE (faster, and DVE is idle then)
                    on_pool = GPSIMD_DRAIN and qb != NQ - 1 and nh == 0

                    def u(tsl=tsl, nh=nh, on_pool=on_pool):
                        ps_o = mmpsum.tile([128, 512], FP32, name="ps",
                                           tag="ps")
                        for hc in range(2):
                            nc.tensor.matmul(
                                ps_o[:, :],
                                lhsT=ctxT[:, hc, tsl],
                                rhs=wo_bf[:, hc, 512 * nh:512 * (nh + 1)],
                                start=(hc == 0), stop=(hc == 1),
                            )
                        o_sb = opool.tile([128, 512], BF16, name="o_sb")
                        if on_pool:
                            nc.gpsimd.tensor_copy(o_sb[:, :], ps_o[:, :])
                        else:
                            nc.vector.tensor_copy(o_sb[:, :], ps_o[:, :])
                        nc.sync.dma_start(
                            out=out_d[tsl, 512 * nh:512 * (nh + 1)],
                            in_=o_sb[:, :])
                    units.append(u)
            return units

        def emit_body():
            # x block 0 first: the first projection unit only needs x0 + wq
            emit_xt_dma(0)
            emit_weights_qkv()
            emit_xt_dma(1)
            emit_weights_o()
            # proj(0) must be complete before the chunk stream starts
            for u in proj_units(0):
                u()

            # ---- software-pipelined main stream ----
            # chunk stream positions: (qb, p, kc) in order
            stream = [(qb, p, kc)
                      for qb in range(NQ) for p in range(2)
                      for kc in range(4 * (qb + 1))]
            group_start = {}
            pos = 0
            for qb in range(NQ):
                for p in range(2):
                    group_start[(qb, p)] = pos
                    pos += 4 * (qb + 1)

            # filler schedule: (ready_pos, order, emit_fn)
            pend = []

            def add(ready, fn):
                pend.append([ready, len(pend), fn])

            # projections of blocks 1..3, spread so block ib drains before
            # its attention group starts
            for ib, (lo, hi) in ((1, (0, 7)), (2, (8, 22)), (3, (24, 46))):
                us = proj_units(ib)
                for j, u in enumerate(us):
                    add(lo + j * max(1, (hi - lo) // len(us)), u)

            # xT DMAs for blocks 2,3 a little ahead of their proj fillers
            dma_at = {4: 2, 16: 3}

            pts_map = {}
            trs_map = {}
            t_ready = {}
            for qb in range(NQ):
                for p in range(2):
                    g = group_start[(qb, p)]
                    pts_map[(qb, p)] = []
                    trs_map[(qb, p)] = {}
                    last = 0
                    for u in range(4):
                        ready = g + 4 * qb + u + 2
                        add(ready, (lambda qb=qb, p=p, u=u:
                                    pv_unit(qb, p, u, pts_map[(qb, p)],
                                            trs_map[(qb, p)])))
                        last = ready
                    add(last + 2, (lambda qb=qb, p=p:
                                   t_unit(qb, p, trs_map[(qb, p)])))
                    t_ready[(qb, p)] = last + 2
            # all output projections go into the filler-starved qb=3 chunk
            # region (positions 48..79, where proj fillers have run dry and
            # ScalarE paces the stream); qb=3's own outproj trails the stream
            j = 0
            for qb in range(NQ - 1):
                for u in outproj_units(qb):
                    add(max(t_ready[(qb, 1)] + 1, 48 + (5 * j) // 4), u)
                    j += 1
            for j, u in enumerate(outproj_units(NQ - 1)):
                add(t_ready[(NQ - 1, 1)] + 1 + j, u)

            pend.sort(key=lambda e: (e[0], e[1]))
            pi = 0
            for posi, (qb, p, kc) in enumerate(stream):
                if posi in dma_at:
                    emit_xt_dma(dma_at[posi])
                while pi < len(pend) and pend[pi][0] <= posi:
                    pend[pi][2]()
                    pi += 1
                emit_chunk(qb, p, kc, pts_map[(qb, p)])
            while pi < len(pend):
                pend[pi][2]()
                pi += 1

        for _rep in range(repeat):
            emit_body()

    nc.compile()
    return nc


def _get_nc(repeat=1):
    key = ("nc", repeat)
    if key not in _CACHE:
        _CACHE[key] = _build_nc(repeat)
    return _CACHE[key]


def _make_in_maps(x, Wq, Wk, Wv, Wo):
    import ml_dtypes
    bf = ml_dtypes.bfloat16
    in_maps = []
    for c in range(NCORES):
        b, g = divmod(c, 4)
        cs = slice(DG * g, DG * (g + 1))
        xb = np.ascontiguousarray(x[b].T).astype(bf)
        in_maps.append({
            "x": xb,
            "wq": np.ascontiguousarray(Wq[:, cs]).astype(bf),
            "wk": np.ascontiguousarray(Wk[:, cs]).astype(bf),
            "wv": np.ascontiguousarray(Wv[:, cs]).astype(bf),
            "wo": np.ascontiguousarray(Wo[cs, :]).astype(bf),
        })
    return in_maps


def _gather(results, bo):
    out = np.empty((2, N, D), dtype=np.float32)
    for b in range(2):
        acc = results[4 * b]["out"].astype(np.float32)
        for g in range(1, 4):
            acc = acc + results[4 * b + g]["out"]
        out[b] = acc + bo[None, :].astype(np.float32)
    return out


def run_spmd(x, Wq, Wk, Wv, Wo, bo, **spmd_kwargs):
    """Run the 8-core kernel; returns (full_output, BassKernelResults)."""
    from concourse.bass_utils import run_bass_kernel_spmd

    nc = _get_nc()
    in_maps = _make_in_maps(
        np.asarray(x), np.asarray(Wq), np.asarray(Wk), np.asarray(Wv),
        np.asarray(Wo))
    res = run_bass_kernel_spmd(nc, in_maps, core_ids=list(range(NCORES)),
                               **spmd_kwargs)
    return _gather(res.results, np.asarray(bo)), res


def kernel(x, Wq, Wk, Wv, Wo, bo):
    out, _ = run_spmd(x, Wq, Wk, Wv, Wo, bo)
    return out
